# revision 5
# baseline (speedup 1.0000x reference)
"""GCN (2-layer, 100K nodes) as a Bass/Tile kernel on 8 trn2 cores — v3.

Math: out = A_n @ relu(A_n @ (x@W1) + b1) @ W2 + b2, A_n = D^-1/2 (A+I) D^-1/2.

Key idea vs v1/v2: the per-chunk DVE one-hot mask (tensor_scalar) serializes
against SWDGE descriptor generation (shared SBUF port pair: a DVE 2-port op
locks GpSimd out, starving the gather DMAs). v3 makes most aggregation
matmuls use CONSTANT masks:

  - Each (dst, src-range) bucket gets K0=5 fixed slots; slot s of a
    superblock-range region maps to dst s//K0. The one-hot mask for chunk c
    is then the data-independent staircase ((128c+j)//K0 == d) — 2*K0=10
    constant [128,256] f16 tiles, loaded once.
  - Edges beyond K0 per (dst,range) go to a small overflow region handled by
    the old DVE-mask path (~9 chunks/superblock vs 44 — 5x less DVE).
  - Empty slots gather a guaranteed-zero table row (nodes are relabeled so
    every core has 44 zero pad rows; index 12500 within each range).
  - norm factorization: dinv[src] is folded into the gathered tables
    (h' = (x@W1)*dinv, z' = z1*dinv written by the epilogues); dinv[dst] is
    applied per-partition after a PE transpose (layer 1, with
    relu(a*x)=a*relu(x) for a>=0) or on the host (layer 2, final output).
  - Layer 2 aggregates z' directly and applies W2 AFTER aggregation
    (A@(z@W2) == (A@z)@W2), so both layers gather 256B rows and the output
    epilogue is a tiny [128f,128d]^T @ W2[128f,32c] matmul.
  - GEMM1 runs in bf16, one [128,512] DMA per row-block.

Sharding: nodes row-sharded (12500 real + 44 zero pad rows per core); edges
partitioned by destination superblock (256 dst); weights replicated;
transformed features all-gathered.
"""

import sys

sys.path.insert(0, "/opt/trn_rl_repo")

import numpy as np
import ml_dtypes

import concourse.bass as bass
import concourse.bacc as bacc
import concourse.mybir as mybir
import concourse.tile as tile
from concourse.bass_utils import run_bass_kernel_spmd
from concourse.library_config import mlp as _mlp_lib

F32 = mybir.dt.float32
F16 = mybir.dt.float16
BF16 = mybir.dt.bfloat16
I16 = mybir.dt.int16

N_CORES = 8
P = 128
SB = 256          # dst nodes per superblock
NRANGE = 4        # src index ranges (so indices fit int16)
GROUP = 4         # superblocks per dma_gather call
NQ = 4            # SWDGE queues
K0 = 5            # const slots per (dst, range)
REAL = 12500      # real nodes per core (rest of the 12544 rows are zero)
ZIDX = 12500      # within-range index of a guaranteed-zero row


def _dims(n_nodes):
    assert n_nodes == N_CORES * REAL
    per = 12544
    np_pad = N_CORES * per
    nsb = np_pad // SB
    bspc = nsb // N_CORES
    return nsb, np_pad, bspc, per


def preprocess(x, edge_index, W1, b1, W2, b2, n_nodes=None):
    n_nodes = n_nodes if n_nodes is not None else x.shape[0]
    in_f = x.shape[1]
    hid = W1.shape[1]
    ncls = W2.shape[1]
    nsb, np_pad, bspc, per = _dims(n_nodes)
    RL = np_pad // NRANGE
    assert RL <= 32767

    loops = np.arange(n_nodes, dtype=np.int64)
    src_o = np.concatenate([np.asarray(edge_index[0], dtype=np.int64), loops])
    dst_o = np.concatenate([np.asarray(edge_index[1], dtype=np.int64), loops])

    deg = np.bincount(dst_o, minlength=n_nodes).astype(np.float32)
    dinv_o = np.zeros(n_nodes, np.float32)
    nz = deg > 0
    dinv_o[nz] = 1.0 / np.sqrt(deg[nz])

    # relabel: core c gets original nodes [REAL*c, REAL*(c+1)) at rows
    # [per*c, per*c+REAL); rows per*c+REAL .. per*(c+1) stay zero.
    def relab(n):
        return per * (n // REAL) + (n % REAL)

    src = relab(src_o)
    dst = relab(dst_o)
    dinv = np.zeros(np_pad, np.float32)
    dinv[relab(np.arange(n_nodes))] = dinv_o

    E_all = len(src)
    sb = dst >> 8
    dloc = dst & (SB - 1)
    rr = src // RL
    srcw = (src - rr * RL).astype(np.int16)

    # --- const slots: first K0 edges of each (dst, range) bucket ---
    kd = dst * NRANGE + rr
    order_d = np.argsort(kd, kind="stable")
    kd_s = kd[order_d]
    cnt_kd = np.bincount(kd_s, minlength=np_pad * NRANGE)
    starts_kd = np.concatenate([[0], np.cumsum(cnt_kd)])
    rank_d = np.arange(E_all) - starts_kd[kd_s]
    const_m = rank_d < K0
    e_c = order_d[const_m]
    slot_c = dloc[e_c] * K0 + rank_d[const_m]          # within range const region

    # --- overflow stream per (sb, range) ---
    e_o = order_d[~const_m]
    ko = sb[e_o] * NRANGE + rr[e_o]
    order_o = np.argsort(ko, kind="stable")
    e_o = e_o[order_o]
    ko_s = ko[order_o]
    cnt_o = np.bincount(ko_s, minlength=nsb * NRANGE)
    starts_o = np.concatenate([[0], np.cumsum(cnt_o)])
    rank_o = np.arange(len(e_o)) - starts_o[ko_s]
    oS_r = np.ceil(cnt_o.reshape(nsb, NRANGE).max(axis=0) / P).astype(int)

    Cr = [2 * K0 + int(oS_r[r]) for r in range(NRANGE)]   # chunks per range
    cumC = np.concatenate([[0], np.cumsum(Cr)]).astype(np.int64)
    SC = int(cumC[-1])                                    # chunks per superblock
    cumO = np.concatenate([[0], np.cumsum(oS_r)]).astype(np.int64)
    OC = int(cumO[-1])                                    # overflow chunks per sb

    slot_o = 2 * K0 * P + rank_o                          # within range region

    srcw_all = np.full(nsb * SC * P, ZIDX, np.int16)
    flat_c = sb[e_c] * (SC * P) + cumC[rr[e_c]] * P + slot_c
    flat_o = sb[e_o] * (SC * P) + cumC[rr[e_o]] * P + slot_o
    srcw_all[flat_c] = srcw[e_c]
    srcw_all[flat_o] = srcw[e_o]
    srcw_all = srcw_all.reshape(nsb, SC * P)

    # overflow dstloc metadata (+2 cols of dinv halves)
    dstloc_ov = np.full((nsb, max(OC, 1) * P), 2.0 * SB, np.float32)
    ovpos = cumO[rr[e_o]] * P + rank_o
    dstloc_ov[sb[e_o], ovpos] = dloc[e_o]
    mdt = np.empty((nsb, P, OC + 2), np.float32)
    mdt[:, :, :OC] = dstloc_ov.reshape(nsb, OC, P).transpose(0, 2, 1)
    dv = dinv.reshape(nsb, 2, P)
    mdt[:, :, OC] = dv[:, 0]
    mdt[:, :, OC + 1] = dv[:, 1]

    # per-range wrapped int16 index arrays: [nsb, 128, Cr*8]
    idx_r = []
    for r in range(NRANGE):
        part = srcw_all[:, cumC[r] * P : cumC[r + 1] * P]
        wrapped = part.reshape(nsb, Cr[r] * 8, 16).transpose(0, 2, 1)
        idx_r.append(np.ascontiguousarray(np.tile(wrapped, (1, 8, 1))))

    # constant staircase masks [128, 2*K0*256] f16
    cm = np.zeros((2 * K0, P, SB), np.float16)
    cc, jj = np.meshgrid(np.arange(2 * K0), np.arange(P), indexing="ij")
    cm[cc, jj, (cc * P + jj) // K0] = 1.0
    cmsk = np.ascontiguousarray(cm.transpose(1, 0, 2).reshape(P, 2 * K0 * SB))

    xpad = np.zeros((np_pad, in_f), np.float32)
    xpad[relab(np.arange(n_nodes))] = x
    kb_n = in_f // P
    nb_n = per // P
    xti = (
        xpad.reshape(np_pad // P, P, kb_n, P)
        .transpose(3, 0, 2, 1)
        .astype(ml_dtypes.bfloat16)
    )  # [128, np_pad/128, kb_n, 128]

    W1c = np.ascontiguousarray(W1, dtype=ml_dtypes.bfloat16)
    W2c = np.ascontiguousarray(W2, dtype=np.float16)
    b1bc = np.tile(np.asarray(b1, np.float16).reshape(1, hid), (P, 1))
    iota = np.tile(np.arange(SB, dtype=np.float16), (P, 1))
    ident = np.eye(P, dtype=np.float16)
    dgc = dinv.reshape(N_CORES, nb_n, P)  # per-core GEMM1 row scales

    in_maps = []
    for c in range(N_CORES):
        blks = slice(c * bspc, (c + 1) * bspc)
        nbs = slice(c * nb_n, (c + 1) * nb_n)
        m = {
            "xti": np.ascontiguousarray(xti[:, nbs].reshape(P, nb_n * kb_n * P)),
            "W1": W1c,
            "W2": W2c,
            "b1bc": b1bc,
            "iota": iota,
            "ident": ident,
            "cmsk": cmsk,
            "dg": np.ascontiguousarray(dgc[c].T),   # [P, nb_n]
            "md": np.ascontiguousarray(mdt[blks]),
        }
        for r in range(NRANGE):
            m[f"idx{r}"] = np.ascontiguousarray(
                idx_r[r][blks].transpose(1, 0, 2).reshape(P, bspc * Cr[r] * 8)
            )
        in_maps.append(m)

    meta = dict(
        n_nodes=n_nodes, in_f=in_f, hid=hid, ncls=ncls,
        nsb=nsb, np_pad=np_pad, bspc=bspc, per=per, RL=RL,
        Cr=tuple(Cr), SC=SC, OC=OC, cumO=tuple(int(v) for v in cumO),
        b2=np.asarray(b2, dtype=np.float32),
        dinv_o=dinv_o,
    )
    return in_maps, (K0, tuple(Cr)), meta


def build_program(S_key, meta, reps=1, timing_variant=False, ablate=()):
    in_f = meta["in_f"]
    hid = meta["hid"]
    ncls = meta["ncls"]
    bspc = meta["bspc"]
    per = meta["per"]
    np_pad = meta["np_pad"]
    RL = meta["RL"]
    Cr = list(meta["Cr"])
    SC = meta["SC"]
    OC = meta["OC"]
    cumO = list(meta["cumO"])
    kb_n = in_f // P
    nb_n = per // P

    nc = bacc.Bacc(
        "TRN2", target_bir_lowering=False, debug=False,
        num_devices=1 if timing_variant else N_CORES,
        num_swdge_queues=NQ,
    )

    xti = nc.dram_tensor("xti", [P, nb_n * kb_n * P], BF16, kind="ExternalInput")
    W1 = nc.dram_tensor("W1", [in_f, hid], BF16, kind="ExternalInput")
    W2 = nc.dram_tensor("W2", [hid, ncls], F16, kind="ExternalInput")
    b1bc = nc.dram_tensor("b1bc", [P, hid], F16, kind="ExternalInput")
    iota = nc.dram_tensor("iota", [P, SB], F16, kind="ExternalInput")
    ident = nc.dram_tensor("ident", [P, P], F16, kind="ExternalInput")
    cmskd = nc.dram_tensor("cmsk", [P, 2 * K0 * SB], F16, kind="ExternalInput")
    dg = nc.dram_tensor("dg", [P, nb_n], F32, kind="ExternalInput")
    md = nc.dram_tensor("md", [bspc, P, OC + 2], F32, kind="ExternalInput")
    idxr = [
        nc.dram_tensor(f"idx{r}", [P, bspc * Cr[r] * 8], I16, kind="ExternalInput")
        for r in range(NRANGE)
    ]
    out = nc.dram_tensor("out", [per, ncls], F32, kind="ExternalOutput")

    groups = [list(range(N_CORES))]
    AL = mybir.AluOpType
    AF = mybir.ActivationFunctionType

    with tile.TileContext(nc) as tc:
        nc.gpsimd.load_library(_mlp_lib)
        with (
            tc.tile_pool(name="const", bufs=1) as const,
            tc.tile_pool(name="dram", bufs=1, space="DRAM") as dram,
            tc.tile_pool(name="xtp", bufs=4) as sb_x,
            tc.tile_pool(name="msgp", bufs=1) as sb_msg,
            tc.tile_pool(name="maskp", bufs=10) as sb_mask,
            tc.tile_pool(name="metap", bufs=4) as sb_meta,
            tc.tile_pool(name="outp", bufs=4) as sb_out,
            tc.tile_pool(name="psum", bufs=3, space="PSUM") as ps,
            tc.tile_pool(name="psum2", bufs=2, space="PSUM") as ps2,
        ):
            w1t = []
            for kb in range(kb_n):
                w = const.tile([P, hid], BF16, tag=f"w1_{kb}")
                nc.sync.dma_start(out=w[:], in_=W1[kb * P : (kb + 1) * P, :])
                w1t.append(w)
            w2t = const.tile([P, ncls], F16, tag="w2")
            nc.sync.dma_start(out=w2t[:], in_=W2[:, :])
            b1t = const.tile([P, hid], F16, tag="b1t")
            nc.sync.dma_start(out=b1t[:], in_=b1bc[:, :])
            iot = const.tile([P, SB], F16, tag="iota")
            nc.sync.dma_start(out=iot[:], in_=iota[:, :])
            idt = const.tile([P, P], F16, tag="ident")
            nc.sync.dma_start(out=idt[:], in_=ident[:, :])
            cmt = const.tile([P, 2 * K0 * SB], F16, tag="cmsk")
            nc.sync.dma_start(out=cmt[:], in_=cmskd[:, :])
            dgt = const.tile([P, nb_n], F32, tag="dg")
            nc.sync.dma_start(out=dgt[:], in_=dg[:, :])

            h_self = dram.tile([per, hid], F16, tag="hself")
            h_full = dram.tile([np_pad, hid], F16, tag="hfull")
            z_self = dram.tile([per, hid], F16, tag="zself")
            z_full = dram.tile([np_pad, hid], F16, tag="zfull")

            def agg_phase(layer):
                table = h_full if layer == 1 else z_full
                elem = hid
                for g0 in range(0, bspc, GROUP):
                    gg = min(GROUP, bspc - g0)
                    msgs = []
                    for r in range(NRANGE):
                        w = Cr[r] * 8
                        idxt = sb_meta.tile([P, gg * w], I16, tag=f"idxt{r}")
                        nc.sync.dma_start(
                            out=idxt[:], in_=idxr[r][:, g0 * w : (g0 + gg) * w]
                        )
                        mt = sb_msg.tile([P, gg * Cr[r] * elem], F16, tag=f"m_{r}")
                        nidx = gg * Cr[r] * P
                        if "gather" in ablate:
                            nc.sync.dma_start(
                                out=mt[:],
                                in_=table[0 : P * gg * Cr[r], :].rearrange(
                                    "(p k) e -> p (k e)", p=P
                                ),
                            )
                        else:
                            nc.gpsimd.dma_gather(
                                mt[:].rearrange("p (c e) -> p c e", e=elem),
                                table[r * RL : (r + 1) * RL, :],
                                idxt[:],
                                nidx,
                                nidx,
                                elem,
                                single_packet=False,
                                queue_num=r % NQ,
                            )
                        msgs.append(mt)
                    for j in range(gg):
                        sb = g0 + j
                        mdt_t = sb_meta.tile([P, OC + 2], F32, tag="mdt")
                        nc.sync.dma_start(out=mdt_t[:], in_=md[sb])
                        acc = ps.tile([P, SB], F32, tag="accA")
                        k = 0
                        for r in range(NRANGE):
                            for c in range(Cr[r]):
                                base = (j * Cr[r] + c) * elem
                                lhs = msgs[r][:, base : base + hid]
                                if c < 2 * K0 or "mask" in ablate:
                                    cc = c if c < 2 * K0 else 0
                                    rhs = cmt[:, cc * SB : (cc + 1) * SB]
                                else:
                                    oc = cumO[r] + (c - 2 * K0)
                                    mask = sb_mask.tile([P, SB], F16, tag="mask")
                                    nc.vector.tensor_scalar(
                                        out=mask[:], in0=iot[:],
                                        scalar1=mdt_t[:, oc : oc + 1],
                                        scalar2=None,
                                        op0=AL.is_equal,
                                    )
                                    rhs = mask[:]
                                if "mm" not in ablate or k in (0, SC - 1):
                                    nc.tensor.matmul(
                                        acc[:], lhsT=lhs, rhs=rhs,
                                        start=(k == 0), stop=(k == SC - 1),
                                    )
                                k += 1
                        csb = sb_out.tile([P, SB], F16, tag="csb")
                        nc.scalar.copy(out=csb[:], in_=acc[:])
                        if layer == 1:
                            for half in range(SB // P):
                                ptr = ps2.tile([P, P], F16, tag="ptp")
                                nc.tensor.transpose(
                                    ptr[:],
                                    csb[:, half * P : (half + 1) * P],
                                    idt[:],
                                )
                                dv = mdt_t[:, OC + half : OC + half + 1]
                                t = sb_out.tile([P, P], F16, tag="tsb")
                                nc.vector.scalar_tensor_tensor(
                                    out=t[:], in0=ptr[:], scalar=dv,
                                    in1=b1t[:, :],
                                    op0=AL.mult, op1=AL.add,
                                )
                                zrow = sb_out.tile([P, P], F16, tag="zrow")
                                nc.scalar.activation(
                                    out=zrow[:], in_=t[:],
                                    func=AF.Relu, bias=0.0, scale=dv,
                                )
                                row = sb * SB + half * P
                                nc.sync.dma_start(
                                    out=z_self[row : row + P, :], in_=zrow[:]
                                )
                        else:
                            for half in range(SB // P):
                                op2 = ps2.tile([P, ncls], F32, tag="op2")
                                nc.tensor.matmul(
                                    op2[:],
                                    lhsT=csb[:, half * P : (half + 1) * P],
                                    rhs=w2t[:],
                                    start=True, stop=True,
                                )
                                osb = sb_out.tile([P, ncls], F32, tag="osb")
                                nc.scalar.copy(out=osb[:], in_=op2[:])
                                row = sb * SB + half * P
                                nc.sync.dma_start(
                                    out=out[row : row + P, :], in_=osb[:]
                                )

            for _rep in range(reps):
                # ---- GEMM1: h' = (x @ W1) * dinv (own shard), bf16 ----
                for nb in range(nb_n):
                    xt = sb_x.tile([P, kb_n * P], BF16, tag="xt")
                    nc.sync.dma_start(
                        out=xt[:],
                        in_=xti[:, nb * kb_n * P : (nb + 1) * kb_n * P],
                    )
                    ph = ps.tile([P, hid], F32, tag="accA")
                    for kb in range(kb_n):
                        nc.tensor.matmul(
                            ph[:], lhsT=xt[:, kb * P : (kb + 1) * P],
                            rhs=w1t[kb][:],
                            start=(kb == 0), stop=(kb == kb_n - 1),
                        )
                    hsb = sb_out.tile([P, hid], F16, tag="hsb")
                    nc.scalar.mul(out=hsb[:], in_=ph[:], mul=dgt[:, nb : nb + 1])
                    nc.sync.dma_start(
                        out=h_self[nb * P : (nb + 1) * P, :], in_=hsb[:]
                    )

                if timing_variant or "cc" in ablate:
                    nc.sync.dma_start(out=h_full[0:per, :], in_=h_self[:])
                else:
                    nc.gpsimd.collective_compute(
                        "AllGather", mybir.AluOpType.bypass, replica_groups=groups,
                        ins=[h_self.opt()], outs=[h_full.opt()],
                    )

                agg_phase(layer=1)

                if timing_variant or "cc" in ablate:
                    nc.sync.dma_start(out=z_full[0:per, :], in_=z_self[:])
                else:
                    nc.gpsimd.collective_compute(
                        "AllGather", mybir.AluOpType.bypass, replica_groups=groups,
                        ins=[z_self.opt()], outs=[z_full.opt()],
                    )

                agg_phase(layer=2)

    nc.compile()
    return nc


_CACHE = {}


def get_program(S_key, meta, reps=1):
    key = (S_key, meta["n_nodes"], meta["in_f"], reps)
    if key not in _CACHE:
        _CACHE[key] = build_program(S_key, meta, reps=reps)
    return _CACHE[key]


def assemble(results, meta):
    n_nodes = meta["n_nodes"]
    ncls = meta["ncls"]
    per = meta["per"]
    full = np.concatenate([results[c]["out"] for c in range(N_CORES)], axis=0)
    rows = (per * (np.arange(n_nodes) // REAL)) + (np.arange(n_nodes) % REAL)
    out = full[rows].astype(np.float32)
    out *= meta["dinv_o"][:, None]
    return out + meta["b2"].reshape(1, ncls)


def kernel(x, edge_index, W1, b1, W2, b2):
    x = np.asarray(x)
    edge_index = np.asarray(edge_index)
    in_maps, S_key, meta = preprocess(x, edge_index, W1, b1, W2, b2)
    nc = get_program(S_key, meta, reps=1)
    res = run_bass_kernel_spmd(nc, in_maps, list(range(N_CORES)))
    return assemble(res.results, meta)


# ---------------------------------------------------------------------------
# Benchmarking helpers.
# ---------------------------------------------------------------------------

def _make_runner(nc, in_maps):
    import jax
    from jax.sharding import Mesh, PartitionSpec
    from jax.experimental.shard_map import shard_map
    from concourse import bass2jax

    bass2jax.install_neuronx_cc_hook()
    partition_name = nc.partition_id_tensor.name if nc.partition_id_tensor else None
    in_names, out_names, out_avals, zero_outs = [], [], [], []
    for alloc in nc.m.functions[0].allocations:
        if not isinstance(alloc, mybir.MemoryLocationSet):
            continue
        name = alloc.memorylocations[0].name
        if alloc.kind == "ExternalInput":
            if name != partition_name:
                in_names.append(name)
        elif alloc.kind == "ExternalOutput":
            out_names.append(name)
            shape = tuple(alloc.tensor_shape)
            dtype = mybir.dt.np(alloc.dtype)
            out_avals.append(jax.core.ShapedArray(shape, dtype))
            zero_outs.append(np.zeros(shape, dtype))
    n_params = len(in_names)
    all_in = in_names + out_names + ([partition_name] if partition_name else [])

    def _body(*args):
        operands = list(args)
        if partition_name is not None:
            operands.append(bass2jax.partition_id_tensor())
        outs = bass2jax._bass_exec_p.bind(
            *operands,
            out_avals=tuple(out_avals),
            in_names=tuple(all_in),
            out_names=tuple(out_names),
            lowering_input_output_aliases=(),
            sim_require_finite=False,
            sim_require_nnan=False,
            nc=nc,
        )
        return tuple(outs)

    devices = jax.devices()[:N_CORES]
    mesh = Mesh(np.asarray(devices), ("core",))
    n_outs = len(out_names)
    fn = jax.jit(
        shard_map(
            _body,
            mesh=mesh,
            in_specs=(PartitionSpec("core"),) * (n_params + n_outs),
            out_specs=(PartitionSpec("core"),) * n_outs,
            check_rep=False,
        ),
        keep_unused=True,
    )
    concat_in = [
        np.concatenate([np.asarray(in_maps[c][n]) for c in range(N_CORES)], axis=0)
        for n in in_names
    ]
    concat_zero = [
        np.zeros((N_CORES * z.shape[0], *z.shape[1:]), z.dtype) for z in zero_outs
    ]
    args = [jax.device_put(a) for a in concat_in + concat_zero]

    def run():
        outs = fn(*args)
        jax.block_until_ready(outs)
        return outs

    return run, out_names, out_avals


def _time_runner(run, iters=8):
    import time

    run()
    best = float("inf")
    for _ in range(iters):
        t0 = time.perf_counter()
        run()
        best = min(best, time.perf_counter() - t0)
    return best


def bench_hw_ns(in_maps, S_key, meta, reps_list=(1, 5), iters=10):
    import time

    runners = []
    for r in reps_list:
        nc = get_program(S_key, meta, reps=r)
        run, _, _ = _make_runner(nc, in_maps)
        for _ in range(3):
            run()
        runners.append(run)
    times = {r: [] for r in reps_list}
    for _ in range(iters):
        for r, run in zip(reps_list, runners):
            t0 = time.perf_counter()
            run()
            times[r].append(time.perf_counter() - t0)
    mins = [min(times[r]) * 1e3 for r in reps_list]
    for r, m in zip(reps_list, mins):
        print(f"  reps={r}: min wall {m:.2f} ms")
    slope = (mins[-1] - mins[0]) / (reps_list[-1] - reps_list[0])
    return slope * 1e6


# revision 6
# speedup vs baseline: 1.8285x; 1.8285x over previous
"""GCN (2-layer, 100K nodes) as a Bass/Tile kernel on 8 trn2 cores — v3.

Math: out = A_n @ relu(A_n @ (x@W1) + b1) @ W2 + b2, A_n = D^-1/2 (A+I) D^-1/2.

Key idea vs v1/v2: the per-chunk DVE one-hot mask (tensor_scalar) serializes
against SWDGE descriptor generation (shared SBUF port pair: a DVE 2-port op
locks GpSimd out, starving the gather DMAs). v3 makes most aggregation
matmuls use CONSTANT masks:

  - Each (dst, src-range) bucket gets K0=5 fixed slots; slot s of a
    superblock-range region maps to dst s//K0. The one-hot mask for chunk c
    is then the data-independent staircase ((128c+j)//K0 == d) — 2*K0=10
    constant [128,256] f16 tiles, loaded once.
  - Edges beyond K0 per (dst,range) go to a small overflow region handled by
    the old DVE-mask path (~9 chunks/superblock vs 44 — 5x less DVE).
  - Empty slots gather a guaranteed-zero table row (nodes are relabeled so
    every core has 44 zero pad rows; index 12500 within each range).
  - norm factorization: dinv[src] is folded into the gathered tables
    (h' = (x@W1)*dinv, z' = z1*dinv written by the epilogues); dinv[dst] is
    applied per-partition after a PE transpose (layer 1, with
    relu(a*x)=a*relu(x) for a>=0) or on the host (layer 2, final output).
  - Layer 2 aggregates z' directly and applies W2 AFTER aggregation
    (A@(z@W2) == (A@z)@W2), so both layers gather 256B rows and the output
    epilogue is a tiny [128f,128d]^T @ W2[128f,32c] matmul.
  - GEMM1 runs in bf16, one [128,512] DMA per row-block.

Sharding: nodes row-sharded (12500 real + 44 zero pad rows per core); edges
partitioned by destination superblock (256 dst); weights replicated;
transformed features all-gathered.
"""

import sys

sys.path.insert(0, "/opt/trn_rl_repo")

import numpy as np
import ml_dtypes

import concourse.bass as bass
import concourse.bacc as bacc
import concourse.mybir as mybir
import concourse.tile as tile
from concourse.bass_utils import run_bass_kernel_spmd
from concourse.library_config import mlp as _mlp_lib

F32 = mybir.dt.float32
F16 = mybir.dt.float16
BF16 = mybir.dt.bfloat16
I16 = mybir.dt.int16

N_CORES = 8
P = 128
SB = 256          # dst nodes per superblock
NRANGE = 4        # src index ranges (so indices fit int16)
GROUP = 4         # superblocks per dma_gather call
NQ = 4            # SWDGE queues
K0 = 5            # const slots per (dst, range)
REAL = 12500      # real nodes per core (rest of the 12544 rows are zero)
ZIDX = 12500      # within-range index of a guaranteed-zero row


def _dims(n_nodes):
    assert n_nodes == N_CORES * REAL
    per = 12544
    np_pad = N_CORES * per
    nsb = np_pad // SB
    bspc = nsb // N_CORES
    return nsb, np_pad, bspc, per


def preprocess(x, edge_index, W1, b1, W2, b2, n_nodes=None):
    n_nodes = n_nodes if n_nodes is not None else x.shape[0]
    in_f = x.shape[1]
    hid = W1.shape[1]
    ncls = W2.shape[1]
    nsb, np_pad, bspc, per = _dims(n_nodes)
    RL = np_pad // NRANGE
    assert RL <= 32767

    loops = np.arange(n_nodes, dtype=np.int64)
    src_o = np.concatenate([np.asarray(edge_index[0], dtype=np.int64), loops])
    dst_o = np.concatenate([np.asarray(edge_index[1], dtype=np.int64), loops])

    deg = np.bincount(dst_o, minlength=n_nodes).astype(np.float32)
    dinv_o = np.zeros(n_nodes, np.float32)
    nz = deg > 0
    dinv_o[nz] = 1.0 / np.sqrt(deg[nz])

    # relabel: core c gets original nodes [REAL*c, REAL*(c+1)) at rows
    # [per*c, per*c+REAL); rows per*c+REAL .. per*(c+1) stay zero.
    def relab(n):
        return per * (n // REAL) + (n % REAL)

    src = relab(src_o)
    dst = relab(dst_o)
    dinv = np.zeros(np_pad, np.float32)
    dinv[relab(np.arange(n_nodes))] = dinv_o

    E_all = len(src)
    sb = dst >> 8
    dloc = dst & (SB - 1)
    rr = src // RL
    srcw = (src - rr * RL).astype(np.int16)

    # --- const slots: first K0 edges of each (dst, range) bucket ---
    kd = dst * NRANGE + rr
    order_d = np.argsort(kd, kind="stable")
    kd_s = kd[order_d]
    cnt_kd = np.bincount(kd_s, minlength=np_pad * NRANGE)
    starts_kd = np.concatenate([[0], np.cumsum(cnt_kd)])
    rank_d = np.arange(E_all) - starts_kd[kd_s]
    const_m = rank_d < K0
    e_c = order_d[const_m]
    slot_c = dloc[e_c] * K0 + rank_d[const_m]          # within range const region

    # --- overflow stream per (sb, range) ---
    e_o = order_d[~const_m]
    ko = sb[e_o] * NRANGE + rr[e_o]
    order_o = np.argsort(ko, kind="stable")
    e_o = e_o[order_o]
    ko_s = ko[order_o]
    cnt_o = np.bincount(ko_s, minlength=nsb * NRANGE)
    starts_o = np.concatenate([[0], np.cumsum(cnt_o)])
    rank_o = np.arange(len(e_o)) - starts_o[ko_s]
    oS_r = np.ceil(cnt_o.reshape(nsb, NRANGE).max(axis=0) / P).astype(int)

    Cr = [2 * K0 + int(oS_r[r]) for r in range(NRANGE)]   # chunks per range
    cumC = np.concatenate([[0], np.cumsum(Cr)]).astype(np.int64)
    SC = int(cumC[-1])                                    # chunks per superblock
    cumO = np.concatenate([[0], np.cumsum(oS_r)]).astype(np.int64)
    OC = int(cumO[-1])                                    # overflow chunks per sb

    slot_o = 2 * K0 * P + rank_o                          # within range region

    srcw_all = np.full(nsb * SC * P, ZIDX, np.int16)
    flat_c = sb[e_c] * (SC * P) + cumC[rr[e_c]] * P + slot_c
    flat_o = sb[e_o] * (SC * P) + cumC[rr[e_o]] * P + slot_o
    srcw_all[flat_c] = srcw[e_c]
    srcw_all[flat_o] = srcw[e_o]
    srcw_all = srcw_all.reshape(nsb, SC * P)

    # overflow dstloc metadata (+2 cols of dinv halves)
    dstloc_ov = np.full((nsb, max(OC, 1) * P), 2.0 * SB, np.float32)
    ovpos = cumO[rr[e_o]] * P + rank_o
    dstloc_ov[sb[e_o], ovpos] = dloc[e_o]
    mdt = np.empty((nsb, P, OC + 2), np.float32)
    mdt[:, :, :OC] = dstloc_ov.reshape(nsb, OC, P).transpose(0, 2, 1)
    dv = dinv.reshape(nsb, 2, P)
    mdt[:, :, OC] = dv[:, 0]
    mdt[:, :, OC + 1] = dv[:, 1]

    # per-range wrapped int16 index arrays: [nsb, 128, Cr*8]
    idx_r = []
    for r in range(NRANGE):
        part = srcw_all[:, cumC[r] * P : cumC[r + 1] * P]
        wrapped = part.reshape(nsb, Cr[r] * 8, 16).transpose(0, 2, 1)
        idx_r.append(np.ascontiguousarray(np.tile(wrapped, (1, 8, 1))))

    # constant staircase masks [128, 2*K0*256] f16
    cm = np.zeros((2 * K0, P, SB), np.float16)
    cc, jj = np.meshgrid(np.arange(2 * K0), np.arange(P), indexing="ij")
    cm[cc, jj, (cc * P + jj) // K0] = 1.0
    cmsk = np.ascontiguousarray(cm.transpose(1, 0, 2).reshape(P, 2 * K0 * SB))

    xpad = np.zeros((np_pad, in_f), np.float32)
    xpad[relab(np.arange(n_nodes))] = x
    kb_n = in_f // P
    nb_n = per // P
    xti = (
        xpad.reshape(np_pad // P, P, kb_n, P)
        .transpose(3, 0, 2, 1)
        .astype(ml_dtypes.bfloat16)
    )  # [128, np_pad/128, kb_n, 128]

    W1c = np.ascontiguousarray(W1, dtype=ml_dtypes.bfloat16)
    W2c = np.ascontiguousarray(W2, dtype=np.float16)
    b1bc = np.tile(np.asarray(b1, np.float16).reshape(1, hid), (P, 1))
    iota = np.tile(np.arange(SB, dtype=np.float16), (P, 1))
    ident = np.eye(P, dtype=np.float16)
    dgc = dinv.reshape(N_CORES, nb_n, P)  # per-core GEMM1 row scales

    in_maps = []
    for c in range(N_CORES):
        blks = slice(c * bspc, (c + 1) * bspc)
        nbs = slice(c * nb_n, (c + 1) * nb_n)
        m = {
            "xti": np.ascontiguousarray(xti[:, nbs].reshape(P, nb_n * kb_n * P)),
            "W1": W1c,
            "W2": W2c,
            "b1bc": b1bc,
            "iota": iota,
            "ident": ident,
            "cmsk": cmsk,
            "dg": np.ascontiguousarray(dgc[c].T),   # [P, nb_n]
            "md": np.ascontiguousarray(mdt[blks]),
        }
        for r in range(NRANGE):
            m[f"idx{r}"] = np.ascontiguousarray(
                idx_r[r][blks].transpose(1, 0, 2).reshape(P, bspc * Cr[r] * 8)
            )
        in_maps.append(m)

    meta = dict(
        n_nodes=n_nodes, in_f=in_f, hid=hid, ncls=ncls,
        nsb=nsb, np_pad=np_pad, bspc=bspc, per=per, RL=RL,
        Cr=tuple(Cr), SC=SC, OC=OC, cumO=tuple(int(v) for v in cumO),
        b2=np.asarray(b2, dtype=np.float32),
        dinv_o=dinv_o,
    )
    return in_maps, (K0, tuple(Cr)), meta


def build_program(S_key, meta, reps=1, timing_variant=False, ablate=()):
    in_f = meta["in_f"]
    hid = meta["hid"]
    ncls = meta["ncls"]
    bspc = meta["bspc"]
    per = meta["per"]
    np_pad = meta["np_pad"]
    RL = meta["RL"]
    Cr = list(meta["Cr"])
    SC = meta["SC"]
    OC = meta["OC"]
    cumO = list(meta["cumO"])
    kb_n = in_f // P
    nb_n = per // P

    nc = bacc.Bacc(
        "TRN2", target_bir_lowering=False, debug=False,
        num_devices=1 if timing_variant else N_CORES,
        num_swdge_queues=NQ,
    )

    xti = nc.dram_tensor("xti", [P, nb_n * kb_n * P], BF16, kind="ExternalInput")
    W1 = nc.dram_tensor("W1", [in_f, hid], BF16, kind="ExternalInput")
    W2 = nc.dram_tensor("W2", [hid, ncls], F16, kind="ExternalInput")
    b1bc = nc.dram_tensor("b1bc", [P, hid], F16, kind="ExternalInput")
    iota = nc.dram_tensor("iota", [P, SB], F16, kind="ExternalInput")
    ident = nc.dram_tensor("ident", [P, P], F16, kind="ExternalInput")
    cmskd = nc.dram_tensor("cmsk", [P, 2 * K0 * SB], F16, kind="ExternalInput")
    dg = nc.dram_tensor("dg", [P, nb_n], F32, kind="ExternalInput")
    md = nc.dram_tensor("md", [bspc, P, OC + 2], F32, kind="ExternalInput")
    idxr = [
        nc.dram_tensor(f"idx{r}", [P, bspc * Cr[r] * 8], I16, kind="ExternalInput")
        for r in range(NRANGE)
    ]
    out = nc.dram_tensor("out", [per, ncls], F32, kind="ExternalOutput")

    groups = [list(range(N_CORES))]
    AL = mybir.AluOpType
    AF = mybir.ActivationFunctionType

    with tile.TileContext(nc) as tc:
        nc.gpsimd.load_library(_mlp_lib)
        with (
            tc.tile_pool(name="const", bufs=1) as const,
            tc.tile_pool(name="dram", bufs=1, space="DRAM") as dram,
            tc.tile_pool(name="xtp", bufs=4) as sb_x,
            tc.tile_pool(name="msgp", bufs=2) as sb_msg,
            tc.tile_pool(name="maskp", bufs=10) as sb_mask,
            tc.tile_pool(name="metap", bufs=4) as sb_meta,
            tc.tile_pool(name="outp", bufs=4) as sb_out,
            tc.tile_pool(name="psum", bufs=3, space="PSUM") as ps,
            tc.tile_pool(name="psum2", bufs=2, space="PSUM") as ps2,
        ):
            w1t = []
            for kb in range(kb_n):
                w = const.tile([P, hid], BF16, tag=f"w1_{kb}")
                nc.sync.dma_start(out=w[:], in_=W1[kb * P : (kb + 1) * P, :])
                w1t.append(w)
            w2t = const.tile([P, ncls], F16, tag="w2")
            nc.sync.dma_start(out=w2t[:], in_=W2[:, :])
            b1t = const.tile([P, hid], F16, tag="b1t")
            nc.sync.dma_start(out=b1t[:], in_=b1bc[:, :])
            iot = const.tile([P, SB], F16, tag="iota")
            nc.sync.dma_start(out=iot[:], in_=iota[:, :])
            idt = const.tile([P, P], F16, tag="ident")
            nc.sync.dma_start(out=idt[:], in_=ident[:, :])
            cmt = const.tile([P, 2 * K0 * SB], F16, tag="cmsk")
            nc.sync.dma_start(out=cmt[:], in_=cmskd[:, :])
            dgt = const.tile([P, nb_n], F32, tag="dg")
            nc.sync.dma_start(out=dgt[:], in_=dg[:, :])

            h_self = dram.tile([per, hid], F16, tag="hself")
            h_full = dram.tile([np_pad, hid], F16, tag="hfull")
            z_self = dram.tile([per, hid], F16, tag="zself")
            z_full = dram.tile([np_pad, hid], F16, tag="zfull")

            def agg_phase(layer):
                table = h_full if layer == 1 else z_full
                elem = hid
                for g0 in range(0, bspc, GROUP):
                    gg = min(GROUP, bspc - g0)
                    msgs = []
                    for r in range(NRANGE):
                        w = Cr[r] * 8
                        idxt = sb_meta.tile([P, gg * w], I16, tag=f"idxt{r}")
                        nc.sync.dma_start(
                            out=idxt[:], in_=idxr[r][:, g0 * w : (g0 + gg) * w]
                        )
                        mt = sb_msg.tile([P, gg * Cr[r] * elem], F16, tag=f"m_{r}")
                        nidx = gg * Cr[r] * P
                        if "gather" in ablate:
                            nc.sync.dma_start(
                                out=mt[:],
                                in_=table[0 : P * gg * Cr[r], :].rearrange(
                                    "(p k) e -> p (k e)", p=P
                                ),
                            )
                        else:
                            nc.gpsimd.dma_gather(
                                mt[:].rearrange("p (c e) -> p c e", e=elem),
                                table[r * RL : (r + 1) * RL, :],
                                idxt[:],
                                nidx,
                                nidx,
                                elem,
                                single_packet=False,
                                queue_num=r % NQ,
                            )
                        msgs.append(mt)
                    for j in range(gg):
                        sb = g0 + j
                        mdt_t = sb_meta.tile([P, OC + 2], F32, tag="mdt")
                        nc.sync.dma_start(out=mdt_t[:], in_=md[sb])
                        acc = ps.tile([P, SB], F32, tag="accA")
                        k = 0
                        for r in range(NRANGE):
                            for c in range(Cr[r]):
                                base = (j * Cr[r] + c) * elem
                                lhs = msgs[r][:, base : base + hid]
                                if c < 2 * K0 or "mask" in ablate:
                                    cc = c if c < 2 * K0 else 0
                                    rhs = cmt[:, cc * SB : (cc + 1) * SB]
                                else:
                                    oc = cumO[r] + (c - 2 * K0)
                                    mask = sb_mask.tile([P, SB], F16, tag="mask")
                                    nc.vector.tensor_scalar(
                                        out=mask[:], in0=iot[:],
                                        scalar1=mdt_t[:, oc : oc + 1],
                                        scalar2=None,
                                        op0=AL.is_equal,
                                    )
                                    rhs = mask[:]
                                if "mm" not in ablate or k in (0, SC - 1):
                                    nc.tensor.matmul(
                                        acc[:], lhsT=lhs, rhs=rhs,
                                        start=(k == 0), stop=(k == SC - 1),
                                    )
                                k += 1
                        csb = sb_out.tile([P, SB], F16, tag="csb")
                        nc.scalar.copy(out=csb[:], in_=acc[:])
                        if layer == 1:
                            for half in range(SB // P):
                                ptr = ps2.tile([P, P], F16, tag="ptp")
                                nc.tensor.transpose(
                                    ptr[:],
                                    csb[:, half * P : (half + 1) * P],
                                    idt[:],
                                )
                                dv = mdt_t[:, OC + half : OC + half + 1]
                                t = sb_out.tile([P, P], F16, tag="tsb")
                                nc.vector.scalar_tensor_tensor(
                                    out=t[:], in0=ptr[:], scalar=dv,
                                    in1=b1t[:, :],
                                    op0=AL.mult, op1=AL.add,
                                )
                                zrow = sb_out.tile([P, P], F16, tag="zrow")
                                nc.scalar.activation(
                                    out=zrow[:], in_=t[:],
                                    func=AF.Relu, bias=0.0, scale=dv,
                                )
                                row = sb * SB + half * P
                                nc.sync.dma_start(
                                    out=z_self[row : row + P, :], in_=zrow[:]
                                )
                        else:
                            for half in range(SB // P):
                                op2 = ps2.tile([P, ncls], F32, tag="op2")
                                nc.tensor.matmul(
                                    op2[:],
                                    lhsT=csb[:, half * P : (half + 1) * P],
                                    rhs=w2t[:],
                                    start=True, stop=True,
                                )
                                osb = sb_out.tile([P, ncls], F32, tag="osb")
                                nc.scalar.copy(out=osb[:], in_=op2[:])
                                row = sb * SB + half * P
                                nc.sync.dma_start(
                                    out=out[row : row + P, :], in_=osb[:]
                                )

            for _rep in range(reps):
                # ---- GEMM1: h' = (x @ W1) * dinv (own shard), bf16 ----
                for nb in range(nb_n):
                    xt = sb_x.tile([P, kb_n * P], BF16, tag="xt")
                    nc.sync.dma_start(
                        out=xt[:],
                        in_=xti[:, nb * kb_n * P : (nb + 1) * kb_n * P],
                    )
                    ph = ps.tile([P, hid], F32, tag="accA")
                    for kb in range(kb_n):
                        nc.tensor.matmul(
                            ph[:], lhsT=xt[:, kb * P : (kb + 1) * P],
                            rhs=w1t[kb][:],
                            start=(kb == 0), stop=(kb == kb_n - 1),
                        )
                    hsb = sb_out.tile([P, hid], F16, tag="hsb")
                    nc.scalar.mul(out=hsb[:], in_=ph[:], mul=dgt[:, nb : nb + 1])
                    nc.sync.dma_start(
                        out=h_self[nb * P : (nb + 1) * P, :], in_=hsb[:]
                    )

                if timing_variant or "cc" in ablate:
                    nc.sync.dma_start(out=h_full[0:per, :], in_=h_self[:])
                else:
                    nc.gpsimd.collective_compute(
                        "AllGather", mybir.AluOpType.bypass, replica_groups=groups,
                        ins=[h_self.opt()], outs=[h_full.opt()],
                    )

                agg_phase(layer=1)

                if timing_variant or "cc" in ablate:
                    nc.sync.dma_start(out=z_full[0:per, :], in_=z_self[:])
                else:
                    nc.gpsimd.collective_compute(
                        "AllGather", mybir.AluOpType.bypass, replica_groups=groups,
                        ins=[z_self.opt()], outs=[z_full.opt()],
                    )

                agg_phase(layer=2)

    nc.compile()
    return nc


_CACHE = {}


def get_program(S_key, meta, reps=1):
    key = (S_key, meta["n_nodes"], meta["in_f"], reps)
    if key not in _CACHE:
        _CACHE[key] = build_program(S_key, meta, reps=reps)
    return _CACHE[key]


def assemble(results, meta):
    n_nodes = meta["n_nodes"]
    ncls = meta["ncls"]
    per = meta["per"]
    full = np.concatenate([results[c]["out"] for c in range(N_CORES)], axis=0)
    rows = (per * (np.arange(n_nodes) // REAL)) + (np.arange(n_nodes) % REAL)
    out = full[rows].astype(np.float32)
    out *= meta["dinv_o"][:, None]
    return out + meta["b2"].reshape(1, ncls)


def kernel(x, edge_index, W1, b1, W2, b2):
    x = np.asarray(x)
    edge_index = np.asarray(edge_index)
    in_maps, S_key, meta = preprocess(x, edge_index, W1, b1, W2, b2)
    nc = get_program(S_key, meta, reps=1)
    res = run_bass_kernel_spmd(nc, in_maps, list(range(N_CORES)))
    return assemble(res.results, meta)


# ---------------------------------------------------------------------------
# Benchmarking helpers.
# ---------------------------------------------------------------------------

def _make_runner(nc, in_maps):
    import jax
    from jax.sharding import Mesh, PartitionSpec
    from jax.experimental.shard_map import shard_map
    from concourse import bass2jax

    bass2jax.install_neuronx_cc_hook()
    partition_name = nc.partition_id_tensor.name if nc.partition_id_tensor else None
    in_names, out_names, out_avals, zero_outs = [], [], [], []
    for alloc in nc.m.functions[0].allocations:
        if not isinstance(alloc, mybir.MemoryLocationSet):
            continue
        name = alloc.memorylocations[0].name
        if alloc.kind == "ExternalInput":
            if name != partition_name:
                in_names.append(name)
        elif alloc.kind == "ExternalOutput":
            out_names.append(name)
            shape = tuple(alloc.tensor_shape)
            dtype = mybir.dt.np(alloc.dtype)
            out_avals.append(jax.core.ShapedArray(shape, dtype))
            zero_outs.append(np.zeros(shape, dtype))
    n_params = len(in_names)
    all_in = in_names + out_names + ([partition_name] if partition_name else [])

    def _body(*args):
        operands = list(args)
        if partition_name is not None:
            operands.append(bass2jax.partition_id_tensor())
        outs = bass2jax._bass_exec_p.bind(
            *operands,
            out_avals=tuple(out_avals),
            in_names=tuple(all_in),
            out_names=tuple(out_names),
            lowering_input_output_aliases=(),
            sim_require_finite=False,
            sim_require_nnan=False,
            nc=nc,
        )
        return tuple(outs)

    devices = jax.devices()[:N_CORES]
    mesh = Mesh(np.asarray(devices), ("core",))
    n_outs = len(out_names)
    fn = jax.jit(
        shard_map(
            _body,
            mesh=mesh,
            in_specs=(PartitionSpec("core"),) * (n_params + n_outs),
            out_specs=(PartitionSpec("core"),) * n_outs,
            check_rep=False,
        ),
        keep_unused=True,
    )
    concat_in = [
        np.concatenate([np.asarray(in_maps[c][n]) for c in range(N_CORES)], axis=0)
        for n in in_names
    ]
    concat_zero = [
        np.zeros((N_CORES * z.shape[0], *z.shape[1:]), z.dtype) for z in zero_outs
    ]
    args = [jax.device_put(a) for a in concat_in + concat_zero]

    def run():
        outs = fn(*args)
        jax.block_until_ready(outs)
        return outs

    return run, out_names, out_avals


def _time_runner(run, iters=8):
    import time

    run()
    best = float("inf")
    for _ in range(iters):
        t0 = time.perf_counter()
        run()
        best = min(best, time.perf_counter() - t0)
    return best


def bench_hw_ns(in_maps, S_key, meta, reps_list=(1, 5), iters=10):
    import time

    runners = []
    for r in reps_list:
        nc = get_program(S_key, meta, reps=r)
        run, _, _ = _make_runner(nc, in_maps)
        for _ in range(3):
            run()
        runners.append(run)
    times = {r: [] for r in reps_list}
    for _ in range(iters):
        for r, run in zip(reps_list, runners):
            t0 = time.perf_counter()
            run()
            times[r].append(time.perf_counter() - t0)
    mins = [min(times[r]) * 1e3 for r in reps_list]
    for r, m in zip(reps_list, mins):
        print(f"  reps={r}: min wall {m:.2f} ms")
    slope = (mins[-1] - mins[0]) / (reps_list[-1] - reps_list[0])
    return slope * 1e6
